# revision 10
# baseline (speedup 1.0000x reference)
"""Trainium2 Bass kernel: cosine-similarity softmin retrieval (DSDM).

reference:  qn = q/||q||; an = a/||a||; sims = qn @ an^T            [B, N]
            w = softmax(10*sims) over N  (softmin of (1-sims)/0.1)
            out = (w @ A)                                           [B, D]

Strategy (8 NeuronCores, flash-attention-style split over N):
  - addresses [200000, 512] sharded row-wise, 25000 rows/core.
  - each core streams its shard once in 128-row tiles (bf16 on-chip, cast
    during the load DMA):
      * row norms ss = sum(a^2) on DVE (affine_mul_reduce)
      * 10/||a|| = exp(-0.5*ln(ss + eps) + ln10) on ACT (one table set)
      * A^T chunks via HWDGE xbar DMA-transpose (bf16, SBUF->SBUF) -- frees
        the PE from 2 of its 3 passes over A and avoids a PSUM->SBUF copy
      * s_raw^T [128j, 64b] = A_chunk @ qn^T via 4 PSUM-accumulated matmuls
      * w^T = Exp(s_raw^T * (10/||a||) - 10) on ACT (fixed shift: cos<=1,
        so logit-10 <= 0; no running max needed)
      * acc [64, 512] += w^T.T @ A in PSUM across all tiles
      * wsum [128, 64] += w^T on GPSIMD; ones-matmul partition-reduce at end
  - host: out = sum_c acc_c / sum_c l_c   (gather/unshard + tiny divide)

Padding: per-core row count 25000 = 195*128 + 40; the last tile's 88 pad
rows are zeroed and get exp bias -40 (weight ~4e-18, exactly negligible).
"""

import math
import os
from collections import OrderedDict

import numpy as np

import concourse.bass as bass
import concourse.tile as tile
from concourse import bacc, mybir
from concourse.bass_utils import run_bass_kernel_spmd
from concourse.masks import make_identity

DT = mybir.dt
AF = mybir.ActivationFunctionType
ALU = mybir.AluOpType

B = 64
D = 512
N_FULL = 200000
NCORES = 8
NPC = N_FULL // NCORES  # 25000
P = 128
G = 4  # tiles per DMA slab
LN10 = math.log(10.0)

# "pe" or "dma": how A^T chunks are produced
TRANSPOSE_MODE = os.environ.get("KERNEL_TRANSPOSE", "pe")
NORMS_MODE = os.environ.get("KERNEL_NORMS", "mixed")
NORM_DVE_OF8 = int(os.environ.get("KERNEL_NORM_DVE_OF8", "4"))  # tiles/8 on DVE
WSUM_MODE = os.environ.get("KERNEL_WSUM", "gpsimd")

LAST_RESULTS = None  # test harness reads exec_time_ns from here


def _patch_act_tables():
    """Prefer the combined natural_log_exp set so Ln/Exp/Square/Copy share
    one ACT table load instead of thrashing 2 loads per slab (~2.7us each)."""
    if getattr(bacc.get_activation_tables, "_patched", False):
        return
    orig = bacc.get_activation_tables

    keep = {AF.Ln, AF.Exp, AF.Square}

    def patched(arch):
        tabs = orig(arch)
        out = OrderedDict()
        for k, fns in tabs.items():
            if k == "natural_log_exp_and_others":
                out[k] = fns
            else:
                out[k] = {f for f in fns if f not in keep}
        return out

    patched._patched = True
    bacc.get_activation_tables = patched


def _build(npc=NPC):
    _patch_act_tables()
    ntiles = (npc + P - 1) // P
    G = max(g for g in range(1, 17) if ntiles % g == 0)  # tiles per slab
    nslabs = ntiles // G
    real_last = npc - (ntiles - 1) * P  # rows in final tile

    nc = bacc.Bacc("TRN2")
    q_d = nc.dram_tensor("query", [B, D], DT.float32, kind="ExternalInput")
    a_d = nc.dram_tensor("addresses", [npc, D], DT.float32, kind="ExternalInput")
    acc_d = nc.dram_tensor("acc", [B, D], DT.float32, kind="ExternalOutput")
    lsum_d = nc.dram_tensor("lsum", [B, 1], DT.float32, kind="ExternalOutput")

    with tile.TileContext(nc) as tc:
        with (
            tc.tile_pool(name="const", bufs=1) as const,
            tc.tile_pool(name="slab", bufs=4) as slab_pool,
            tc.tile_pool(name="at", bufs=3) as at_pool,
            tc.tile_pool(name="wt", bufs=4) as wt_pool,
            tc.tile_pool(name="small", bufs=4) as small,
            tc.tile_pool(name="ps_at", bufs=2, space="PSUM") as ps_at,
            tc.tile_pool(name="ps_s", bufs=3, space="PSUM") as ps_s,
            tc.tile_pool(name="ps_one", bufs=1, space="PSUM") as ps_one,
            tc.tile_pool(name="ps_acc", bufs=1, space="PSUM") as ps_acc,
        ):
            ident = const.tile([P, P], DT.bfloat16)
            make_identity(nc, ident)
            bias_main = const.tile([P, 1], DT.float32)
            nc.vector.memset(bias_main, -10.0)
            bias_last = const.tile([P, 1], DT.float32)
            nc.vector.memset(bias_last, -40.0)
            if real_last > 0:
                nc.vector.memset(bias_last[:real_last], -10.0)
            ones = const.tile([P, 1], DT.float32)
            nc.vector.memset(ones, 1.0)
            eps12 = const.tile([P, 1], DT.float32)
            nc.vector.memset(eps12, 1e-12)
            ln10b = const.tile([P, 1], DT.float32)
            nc.vector.memset(ln10b, LN10)
            wsum = const.tile([P, B], DT.float32)
            nc.vector.memset(wsum, 0.0)

            # ---- query preprocessing: qn^T bf16 chunks [128d, 4c, 64b] ----
            q_sb = const.tile([B, D], DT.float32)
            nc.sync.dma_start(out=q_sb, in_=q_d[:, :])
            qsq = const.tile([B, D], DT.float32)
            ssq = const.tile([B, 1], DT.float32)
            nc.scalar.activation(qsq, q_sb, AF.Square, accum_out=ssq)
            lnq = const.tile([B, 1], DT.float32)
            nc.scalar.activation(lnq, ssq, AF.Ln, bias=eps12[:B])
            invq = const.tile([B, 1], DT.float32)
            nc.scalar.activation(invq, lnq, AF.Exp, scale=-0.5)
            qn = const.tile([B, D], DT.bfloat16)
            nc.vector.tensor_scalar_mul(out=qn, in0=q_sb, scalar1=invq)
            qnT = const.tile([P, 4, B], DT.bfloat16)
            for c in range(4):
                qt_ps = ps_one.tile([P, B], DT.bfloat16, tag="qt")
                nc.tensor.transpose(qt_ps, qn[:, c * P:(c + 1) * P], ident[:B, :B])
                nc.scalar.copy(qnT[:, c, :], qt_ps)

            # ---- main streaming loop ----
            acc_ps = ps_acc.tile([B, D], DT.float32)
            for g in range(nslabs):
                a_sl = slab_pool.tile([P, G, D], DT.bfloat16)
                last_slab = g == nslabs - 1
                if not last_slab or real_last == P:
                    nc.gpsimd.dma_start(
                        out=a_sl,
                        in_=a_d[g * G * P:(g + 1) * G * P, :].rearrange(
                            "(t p) d -> p t d", p=P))
                else:
                    for t in range(G - 1):
                        r0 = (g * G + t) * P
                        nc.gpsimd.dma_start(out=a_sl[:, t, :], in_=a_d[r0:r0 + P, :])
                    nc.gpsimd.memset(a_sl[:, G - 1, :], 0)
                    nc.gpsimd.dma_start(
                        out=a_sl[:real_last, G - 1, :],
                        in_=a_d[(ntiles - 1) * P:npc, :])

                # row norms for the whole slab, then 10/||a|| via ln/exp
                ss = small.tile([P, G], DT.float32)
                for t in range(G):
                    sq = small.tile([P, D], DT.bfloat16, tag="sq")
                    gt0 = g * G + t
                    use_dve = (NORMS_MODE == "amr") or (
                        NORMS_MODE == "mixed" and (gt0 % 8) < NORM_DVE_OF8)
                    if use_dve:
                        nc.vector.affine_mul_reduce(
                            out=sq, accum_out=ss[:, t:t + 1],
                            in0=a_sl[:, t, :], in1=a_sl[:, t, :], scale=1.0, bias=0.0)
                    else:
                        nc.scalar.activation(sq, a_sl[:, t, :], AF.Square,
                                             accum_out=ss[:, t:t + 1])
                lns = small.tile([P, G], DT.float32)
                nc.scalar.activation(lns, ss, AF.Ln, bias=eps12)
                inv = small.tile([P, G], DT.float32)
                nc.scalar.activation(inv, lns, AF.Exp, scale=-0.5, bias=ln10b)

                for t in range(G):
                    gt = g * G + t
                    at_sb = at_pool.tile([P, 4, P], DT.bfloat16)
                    if TRANSPOSE_MODE == "dma":
                        for c in range(4):
                            nc.sync.dma_start(
                                out=at_sb[:, c, :],
                                in_=a_sl[:, t, c * P:(c + 1) * P],
                                transpose=True)
                    else:
                        at_ps = ps_at.tile([P, 4, P], DT.bfloat16)
                        for c in range(4):
                            nc.tensor.transpose(
                                at_ps[:, c, :], a_sl[:, t, c * P:(c + 1) * P], ident)
                        nc.vector.tensor_copy(at_sb, at_ps)
                    s_ps = ps_s.tile([P, B], DT.float32, tag="s")
                    for c in range(4):
                        nc.tensor.matmul(
                            s_ps, lhsT=at_sb[:, c, :], rhs=qnT[:, c, :],
                            start=(c == 0), stop=(c == 3))
                    wt = wt_pool.tile([P, B], DT.bfloat16)
                    nc.scalar.activation(
                        wt, s_ps, AF.Exp,
                        bias=bias_last if gt == ntiles - 1 else bias_main,
                        scale=inv[:, t:t + 1])
                    nc.tensor.matmul(
                        acc_ps, lhsT=wt, rhs=a_sl[:, t, :],
                        start=(gt == 0), stop=(gt == ntiles - 1))
                    if WSUM_MODE == "gpsimd":
                        nc.gpsimd.tensor_add(wsum, wsum, wt)
                    else:
                        nc.vector.tensor_add(wsum, wsum, wt)

            # ---- epilogue: normalizer + writeback ----
            l_ps = ps_one.tile([B, 1], DT.float32, tag="l")
            nc.tensor.matmul(l_ps, lhsT=wsum, rhs=ones)
            acc_sb = const.tile([B, D], DT.float32)
            nc.scalar.copy(acc_sb, acc_ps)
            l_sb = const.tile([B, 1], DT.float32)
            nc.vector.tensor_copy(l_sb, l_ps)
            nc.sync.dma_start(out=acc_d[:, :], in_=acc_sb)
            nc.sync.dma_start(out=lsum_d[:, :], in_=l_sb)

    nc.finalize()
    return nc


_NC_CACHE = {}


def _get_nc(npc=NPC):
    if npc not in _NC_CACHE:
        _NC_CACHE[npc] = _build(npc)
    return _NC_CACHE[npc]


def kernel(query, addresses):
    global LAST_RESULTS
    query = np.ascontiguousarray(np.asarray(query), dtype=np.float32)
    addresses = np.ascontiguousarray(np.asarray(addresses), dtype=np.float32)
    n = addresses.shape[0]
    npc = n // NCORES
    assert npc * NCORES == n
    nc = _get_nc(npc)
    in_maps = [
        {"query": query, "addresses": addresses[c * npc:(c + 1) * npc]}
        for c in range(NCORES)
    ]
    res = run_bass_kernel_spmd(nc, in_maps, core_ids=list(range(NCORES)))
    LAST_RESULTS = res
    acc = np.zeros((B, D), np.float64)
    l = np.zeros((B, 1), np.float64)
    for r in res.results:
        acc += r["acc"].astype(np.float64)
        l += r["lsum"].astype(np.float64)
    return (acc / l).astype(np.float32)
